# revision 6
# baseline (speedup 1.0000x reference)
"""Trainium2 Bass kernel for a 2-layer, 4-attention-block transformer decoder.

Contract: kernel(**inputs) takes FULL inputs (x, encoder_out, pe_out, pd_out,
params) and returns the FULL (2, 1024, 1024) float32 output.

Distribution: 8 NeuronCores = 2 batch groups x 4-way sequence parallel.
Core (4g + c) owns batch g, query chunks {c, 7-c} (128 rows each).
Self/cross-attention K/V are computed position-sharded and AllGather'd
within each 4-core group; everything else is local. Causal attention is
block-causal (low chunk vs 512 keys, high chunk vs 1024 keys) with
per-core additive masks supplied as data, so one SPMD program serves all
cores.
"""

import sys

sys.path.insert(0, "/opt/trn_rl_repo")

import numpy as np
import ml_dtypes

import concourse.bass as bass
import concourse.mybir as mybir
import concourse.tile as tile
from concourse import bacc
from concourse import bass_utils

BF16 = ml_dtypes.bfloat16
F32 = mybir.dt.float32
B16 = mybir.dt.bfloat16

B, S, D, H, L, DF, DK = 2, 1024, 1024, 16, 2, 4096, 64
NCORE = 8
GROUPS = [[0, 1, 2, 3], [4, 5, 6, 7]]
CH = 128          # chunk rows
SO = 256          # owned rows per core
MHAS = ("sa", "pd", "ed", "pe")
NEG = -30000.0

AF = mybir.ActivationFunctionType
ALU = mybir.AluOpType


def _bcast_rows(ap, p):
    """View a (1, N) DRAM AP as (p, N) with partition stride 0 (DMA broadcast)."""
    return bass.AP(tensor=ap.tensor, offset=ap.offset, ap=[[0, p]] + list(ap.ap[1:]))


def _build():
    nc = bacc.Bacc("TRN2", target_bir_lowering=False, debug=False, num_devices=NCORE)

    T = {}

    def din(name, shape, dt):
        T[name] = nc.dram_tensor(name, list(shape), dt, kind="ExternalInput")
        return T[name]

    din("xT", (D, SO), B16)
    din("x_own", (SO, D), F32)
    din("encT", (D, SO), B16)
    din("peT", (D, SO), B16)
    din("pdT", (D, SO), B16)
    din("maskT", (128, 8, SO), B16)
    for l in range(L):
        for m in MHAS:
            din(f"wq_{l}{m}", (D, D), B16)
            din(f"wk_{l}{m}", (D, D), B16)
            din(f"wv_{l}{m}", (D, D), B16)
            din(f"wh_{l}{m}", (128, 512), B16)
            din(f"wo_{l}{m}", (D, D), B16)
            din(f"bq_{l}{m}", (128, 8), F32)
            din(f"bk_{l}{m}", (128, 8), F32)
            din(f"bh_{l}{m}", (128, 8), F32)
            din(f"bv_{l}{m}", (1, D), F32)
            din(f"bo_{l}{m}", (1, D), F32)
        din(f"w1_{l}", (D, DF), B16)
        din(f"b1c_{l}", (128, 32), F32)
        din(f"w2_{l}", (DF, D), B16)
        din(f"b2r_{l}", (1, D), F32)
        for j in (1, 2, 3):
            din(f"g{j}_{l}", (1, D), B16)
            din(f"bb{j}_{l}", (1, D), B16)
    out_d = nc.dram_tensor("out", [SO, D], F32, kind="ExternalOutput")

    with tile.TileContext(nc) as tc:
        _program(nc, tc, T, out_d)

    nc.compile()
    return nc


def _program(nc, tc, T, out_d):
    import contextlib

    ctx = contextlib.ExitStack()
    with ctx:
        ps = ctx.enter_context(tc.tile_pool(name="ps", bufs=8, space="PSUM"))
        wp = ctx.enter_context(tc.tile_pool(name="wp", bufs=9))
        wsm = ctx.enter_context(tc.tile_pool(name="wsm", bufs=3))
        cst = ctx.enter_context(tc.tile_pool(name="cst", bufs=1))
        hTp = ctx.enter_context(tc.tile_pool(name="hTp", bufs=2))
        hxp = ctx.enter_context(tc.tile_pool(name="hxp", bufs=1))
        rTp = ctx.enter_context(tc.tile_pool(name="rTp", bufs=2))
        zxp = ctx.enter_context(tc.tile_pool(name="zxp", bufs=3))
        qTp = ctx.enter_context(tc.tile_pool(name="qTp", bufs=2))
        kTp = ctx.enter_context(tc.tile_pool(name="kTp", bufs=1))
        vtp = ctx.enter_context(tc.tile_pool(name="vtp", bufs=1))
        pTp = ctx.enter_context(tc.tile_pool(name="pTp", bufs=4))
        cxp = ctx.enter_context(tc.tile_pool(name="cxp", bufs=3))
        c2p = ctx.enter_context(tc.tile_pool(name="c2p", bufs=2))
        a1p = ctx.enter_context(tc.tile_pool(name="a1p", bufs=1))
        gbp = ctx.enter_context(tc.tile_pool(name="gbp", bufs=2))
        sev = ctx.enter_context(tc.tile_pool(name="sev", bufs=2))
        seb = ctx.enter_context(tc.tile_pool(name="seb", bufs=4))
        kev = ctx.enter_context(tc.tile_pool(name="kev", bufs=2))
        vev = ctx.enter_context(tc.tile_pool(name="vev", bufs=2))
        b8p = ctx.enter_context(tc.tile_pool(name="b8p", bufs=6))
        brp = ctx.enter_context(tc.tile_pool(name="brp", bufs=2))
        b1p = ctx.enter_context(tc.tile_pool(name="b1p", bufs=2))
        bnp = ctx.enter_context(tc.tile_pool(name="bnp", bufs=4))
        shp = ctx.enter_context(tc.tile_pool(name="shp", bufs=2, space="DRAM"))
        gap = ctx.enter_context(tc.tile_pool(name="gap", bufs=8, space="DRAM"))

        # ---- constants ----
        ident = cst.tile([128, 128], F32, tag="ident")
        from concourse.masks import make_identity

        make_identity(nc, ident[:])
        eps_t = cst.tile([128, 1], F32, tag="eps")
        nc.vector.memset(eps_t[:], 1e-5)
        ones1 = cst.tile([1, 128], F32, tag="ones1")
        nc.vector.memset(ones1[:], 1.0)
        masks = cst.tile([128, 8, SO], B16, tag="masks")
        nc.sync.dma_start(masks[:], T["maskT"].ap())
        srcT_sb = {}
        for nm in ("encT", "peT", "pdT"):
            t = cst.tile([128, 8, SO], B16, tag=nm)
            nc.sync.dma_start(t[:], T[nm].ap().rearrange("(ks p) c -> p ks c", p=128))
            srcT_sb[nm] = t

        def load_w_slices(w_d, n=8):
            """Load (128, 1024) row-slices of a (1024,1024) DRAM weight."""
            ts_ = []
            wv = w_d.ap().rearrange("(ks p) n -> ks p n", p=128)
            for k in range(n):
                wt = wp.tile([128, 1024], B16, tag="w", name=f"w_{k}")
                nc.sync.dma_start(wt[:], wv[k])
                ts_.append(wt)
            return ts_

        def proj_pairs(srcT, w_d, b8_d, outpool, tag):
            """pairs(hp) = (W[:, hp*128:+128]).T @ src  + bias; -> (128, 8, 256) bf16."""
            out_sb = outpool.tile([128, 8, SO], B16, tag=tag)
            b_sb = b8p.tile([128, 8], F32, tag="b8")
            nc.sync.dma_start(b_sb[:], b8_d.ap())
            wt = load_w_slices(w_d)
            pss = []
            for hp in range(8):
                pss.append(ps.tile([128, 512], F32, tag="ps", name=f"ps_pp{hp}"))
            for half in range(2):
                for ks in range(8):
                    for hp in range(4 * half, 4 * half + 4):
                        nc.tensor.matmul(
                            pss[hp][:, :SO],
                            wt[ks][:, hp * 128 : hp * 128 + 128],
                            srcT[:, ks, :],
                            start=(ks == 0),
                            stop=(ks == 7),
                        )
                for hp in range(4 * half, 4 * half + 4):
                    nc.scalar.activation(
                        out_sb[:, hp, :], pss[hp][:, :SO], AF.Identity,
                        bias=b_sb[:, hp : hp + 1],
                    )
            return out_sb

        def proj_natural(srcT, w_d, brow_d):
            """nat = src @ W + brow ; -> (128, 2, 1024) bf16 (positions on partitions)."""
            out_sb = vev.tile([128, 2, 1024], B16, tag="vev")
            br = brp.tile([1, 1024], F32, tag="brow")
            nc.sync.dma_start(br[:], brow_d.ap())
            wt = load_w_slices(w_d)
            pss = [ps.tile([128, 512], F32, tag="ps", name=f"ps_pn{i}") for i in range(4)]
            for ks in range(8):
                for c in range(2):
                    for ns in range(2):
                        nc.tensor.matmul(
                            pss[c * 2 + ns],
                            srcT[:, ks, c * 128 : c * 128 + 128],
                            wt[ks][:, ns * 512 : ns * 512 + 512],
                            start=(ks == 0),
                            stop=False,
                        )
            for c in range(2):
                for ns in range(2):
                    nc.tensor.matmul(
                        pss[c * 2 + ns], ones1[:, :], br[0:1, ns * 512 : ns * 512 + 512],
                        start=False, stop=True, skip_group_check=True,
                    )
                    nc.scalar.activation(
                        out_sb[:, c, ns * 512 : ns * 512 + 512], pss[c * 2 + ns],
                        AF.Identity,
                    )
            return out_sb

        def kv_pass(srcT, l, m):
            """Compute k (pair layout) + v (natural) shards, write to DRAM, AllGather."""
            shard = shp.tile([2048, SO], B16, tag="shard")
            gath = gap.tile([4 * 2048, SO], B16, tag="gath")
            k_sb = proj_pairs(srcT, T[f"wk_{l}{m}"], T[f"bk_{l}{m}"], kev, "kev")
            nc.sync.dma_start(
                shard[:].rearrange("(half hp p) u -> half p hp u", half=2, p=128)[0],
                k_sb[:],
            )
            v_sb = proj_natural(srcT, T[f"wv_{l}{m}"], T[f"bv_{l}{m}"])
            vshard = shard[:].rearrange(
                "(half pos four) u -> half pos (four u)", half=2, four=4
            )[1]
            for c in range(2):
                nc.sync.dma_start(vshard[c * 128 : c * 128 + 128, :], v_sb[:, c, :])
            nc.gpsimd.collective_compute(
                "AllGather", ALU.bypass, replica_groups=GROUPS,
                ins=[shard[:].opt()], outs=[gath[:].opt()],
            )
            return gath

        def attn_stage(l, mhas, qts, gaths, masked, resid_x, zx_out):
            for mi, m in enumerate(mhas):
                gath = gaths[mi]
                qT = qts[mi]
                kT = kTp.tile([128, 8, 1024], B16, tag="kT")
                gv = gath[:].rearrange("(r sub p) u -> p sub r u", r=4, p=128)
                kT4 = kT[:].rearrange("p hp (r u) -> p hp r u", r=4)
                for r in range(4):
                    nc.sync.dma_start(kT4[:, :, r], gv[:, 0:8, r])
                vt = vtp.tile([128, 8, 16, 66], B16, tag="vt")
                nc.vector.memset(vt[:, :, :, 64:65], 1.0)
                vfull = gath[:].rearrange(
                    "(r hh pos four) u -> r hh pos (four u)", r=4, hh=2, four=4
                )
                for kb in range(8):
                    nc.sync.dma_start(
                        vt[:, kb, :, 0:64],
                        vfull[kb // 2, 1, (kb % 2) * 128 : (kb % 2) * 128 + 128, :]
                        .rearrange("p (hh dv) -> p hh dv", dv=64),
                    )
                wh_sb = wsm.tile([128, 512], B16, tag="wsm")
                nc.sync.dma_start(wh_sb[:], T[f"wh_{l}{m}"].ap())
                wh3 = wh_sb[:].rearrange("p (hp e) -> p hp e", e=64)
                bh_sb = b8p.tile([128, 8], F32, tag="b8")
                nc.sync.dma_start(bh_sb[:], T[f"bh_{l}{m}"].ap())
                cat2 = c2p.tile([128, 8, SO], B16, tag="cat2")

                for hp in range(8):
                    ctxn = cxp.tile([128, SO], B16, tag="ctxn")
                    for hi in range(2):
                        h = 2 * hp + hi
                        r0 = 64 * hi
                        ctx_ps = ps.tile([128, 512], F32, tag="ps", name="ps_ctx")
                        for kb in range(8):
                            s_ps = ps.tile([128, 512], F32, tag="ps", name="ps_s")
                            if masked:
                                nc.tensor.matmul(
                                    s_ps[:, 128:256],
                                    kT[r0 : r0 + 64, hp, kb * 128 : kb * 128 + 128],
                                    qT[r0 : r0 + 64, hp, 128:256],
                                    start=True, stop=True, skip_group_check=True,
                                )
                                if kb < 4:
                                    nc.tensor.matmul(
                                        s_ps[:, 0:128],
                                        kT[r0 : r0 + 64, hp, kb * 128 : kb * 128 + 128],
                                        qT[r0 : r0 + 64, hp, 0:128],
                                        start=True, stop=True, skip_group_check=True,
                                    )
                                w0 = 0 if kb < 4 else 128
                                nc.vector.tensor_tensor(
                                    s_ps[:, w0:256], s_ps[:, w0:256],
                                    masks[:, kb, w0:256], ALU.add,
                                )
                            else:
                                w0 = 0
                                nc.tensor.matmul(
                                    s_ps[:, 0:256],
                                    kT[r0 : r0 + 64, hp, kb * 128 : kb * 128 + 128],
                                    qT[r0 : r0 + 64, hp, :],
                                    start=True, stop=True, skip_group_check=True,
                                )
                            pT = pTp.tile([128, SO], B16, tag="pT")
                            nc.scalar.activation(
                                pT[:, w0:256], s_ps[:, w0:256], AF.Exp, scale=0.125
                            )
                            nc.tensor.matmul(
                                ctx_ps[0:65, w0:256],
                                vt[:, kb, h, 0:65],
                                pT[:, w0:256],
                                start=(kb == 0), stop=(kb == 7),
                                skip_group_check=True,
                            )
                        se_t = sev.tile([1, SO], F32, tag="sev")
                        nc.vector.reciprocal(se_t[:], ctx_ps[64:65, 0:256])
                        sb_t = seb.tile([64, SO], F32, tag="seb")
                        nc.gpsimd.partition_broadcast(sb_t[:], se_t[:])
                        nc.vector.tensor_tensor(
                            ctxn[r0 : r0 + 64, :], ctx_ps[0:64, 0:256], sb_t[:],
                            ALU.mult,
                        )
                    c2ps = ps.tile([128, 512], F32, tag="ps", name="ps_c2")
                    for hi in range(2):
                        r0 = 64 * hi
                        nc.tensor.matmul(
                            c2ps[r0 : r0 + 64, 0:256],
                            wh3[r0 : r0 + 64, hp, :],
                            ctxn[r0 : r0 + 64, :],
                            start=True, stop=True, skip_group_check=True,
                        )
                    nc.scalar.activation(
                        cat2[:, hp, :], c2ps[:, 0:256], AF.Identity,
                        bias=bh_sb[:, hp : hp + 1],
                    )

                # Wo (pre-scaled by 0.5 host-side) + bias + accumulate into zx_out
                wo_t = load_w_slices(T[f"wo_{l}{m}"])
                bo_r = brp.tile([1, 1024], F32, tag="brow")
                nc.sync.dma_start(bo_r[:], T[f"bo_{l}{m}"].ap())
                for c in range(2):
                    for ns in range(2):
                        o_ps = ps.tile([128, 512], F32, tag="ps", name="ps_o")
                        for kp in range(8):
                            nc.tensor.matmul(
                                o_ps,
                                cat2[:, kp, c * 128 : c * 128 + 128],
                                wo_t[kp][:, ns * 512 : ns * 512 + 512],
                                start=(kp == 0), stop=False,
                            )
                        nc.tensor.matmul(
                            o_ps, ones1[:, :], bo_r[0:1, ns * 512 : ns * 512 + 512],
                            start=False, stop=True, skip_group_check=True,
                        )
                        sl = slice(ns * 512, ns * 512 + 512)
                        if mi == 0:
                            nc.vector.tensor_tensor(
                                zx_out[:, c, sl], o_ps, resid_x[:, c, sl], ALU.add
                            )
                        else:
                            nc.vector.tensor_tensor(
                                zx_out[:, c, sl], o_ps, zx_out[:, c, sl], ALU.add
                            )

        def ln_stage(z_x, g_d, b_d, transpose_to=None):
            g_b = gbp.tile([128, 1024], B16, tag="gb")
            nc.sync.dma_start(g_b[:], _bcast_rows(g_d.ap(), 128))
            b_b = gbp.tile([128, 1024], B16, tag="gb")
            nc.sync.dma_start(b_b[:], _bcast_rows(b_d.ap(), 128))
            for c in range(2):
                zc = z_x[:, c, :]
                st = bnp.tile([128, 2, 6], F32, tag="bnst")
                for sg in range(2):
                    nc.vector.bn_stats(st[:, sg, :], zc[:, sg * 512 : sg * 512 + 512])
                mv = bnp.tile([128, 2], F32, tag="bnmv")
                nc.vector.bn_aggr(mv[:], st[:])
                rstd = bnp.tile([128, 1], F32, tag="rstd")
                nc.scalar.activation(
                    rstd[:], mv[:, 1:2], AF.Sqrt, bias=eps_t[:, :]
                )
                nc.vector.reciprocal(rstd[:], rstd[:])
                nc.vector.tensor_scalar(
                    zc, zc, mv[:, 0:1], rstd[:, 0:1], ALU.subtract, ALU.mult
                )
                nc.vector.tensor_tensor(zc, zc, g_b[:], ALU.mult)
                nc.vector.tensor_tensor(zc, zc, b_b[:], ALU.add)
            rT = None
            if transpose_to is not None:
                pool, tag = transpose_to
                rT = pool.tile([128, 8, SO], B16, tag=tag)
                for c in range(2):
                    for ks in range(8):
                        tp = ps.tile([128, 512], F32, tag="ps", name="ps_t")
                        nc.tensor.transpose(
                            tp[:, 0:128], z_x[:, c, ks * 128 : ks * 128 + 128], ident[:]
                        )
                        nc.scalar.activation(
                            rT[:, ks, c * 128 : c * 128 + 128], tp[:, 0:128],
                            AF.Identity,
                        )
            return z_x, rT

        def ffn(r2T, l, resid_x, zx_out):
            b1c = b1p.tile([128, 32], F32, tag="b1c")
            nc.sync.dma_start(b1c[:], T[f"b1c_{l}"].ap())
            a1 = a1p.tile([128, 32, SO], B16, tag="a1")
            w1v = T[f"w1_{l}"].ap().rearrange("(ks p) n -> ks p n", p=128)
            for dtg in range(8):
                pss = [ps.tile([128, 512], F32, tag="ps", name=f"ps_pn{i}") for i in range(4)]
                for ks in range(8):
                    wt = wsm.tile([128, 512], B16, tag="wsm")
                    nc.sync.dma_start(
                        wt[:], w1v[ks, :, dtg * 512 : dtg * 512 + 512]
                    )
                    for d4 in range(4):
                        nc.tensor.matmul(
                            pss[d4][:, :SO],
                            wt[:, d4 * 128 : d4 * 128 + 128],
                            r2T[:, ks, :],
                            start=(ks == 0), stop=(ks == 7),
                        )
                for d4 in range(4):
                    dt = dtg * 4 + d4
                    nc.scalar.activation(
                        a1[:, dt, :], pss[d4][:, :SO], AF.Relu,
                        bias=b1c[:, dt : dt + 1],
                    )
            w2v = T[f"w2_{l}"].ap().rearrange("(ks p) n -> ks p n", p=128)
            fps = [ps.tile([128, 512], F32, tag="ps", name=f"ps_ff{i}") for i in range(4)]
            for ks in range(32):
                w2t = wp.tile([128, 1024], B16, tag="w")
                nc.sync.dma_start(w2t[:], w2v[ks])
                for c in range(2):
                    for ns in range(2):
                        nc.tensor.matmul(
                            fps[c * 2 + ns],
                            a1[:, ks, c * 128 : c * 128 + 128],
                            w2t[:, ns * 512 : ns * 512 + 512],
                            start=(ks == 0), stop=False,
                        )
            b2r = brp.tile([1, 1024], F32, tag="brow")
            nc.sync.dma_start(b2r[:], T[f"b2r_{l}"].ap())
            for c in range(2):
                for ns in range(2):
                    nc.tensor.matmul(
                        fps[c * 2 + ns], ones1[:, :],
                        b2r[0:1, ns * 512 : ns * 512 + 512],
                        start=False, stop=True, skip_group_check=True,
                    )
                    sl = slice(ns * 512, ns * 512 + 512)
                    nc.vector.tensor_tensor(
                        zx_out[:, c, sl], fps[c * 2 + ns], resid_x[:, c, sl], ALU.add
                    )

        # ---- program ----
        hT = hTp.tile([128, 8, SO], B16, tag="hT")
        nc.sync.dma_start(hT[:], T["xT"].ap().rearrange("(ks p) c -> p ks c", p=128))
        h_x = hxp.tile([128, 2, 1024], F32, tag="hx")
        nc.sync.dma_start(
            h_x[:], T["x_own"].ap().rearrange("(c p) d -> p c d", p=128)
        )

        # upfront: self-attn l0 KV first (its AG is on the critical path),
        # then fixed cross-attn KV for all layers.
        gath_sa = kv_pass(hT, 0, "sa")
        gfix = {}
        for l in range(L):
            for m, src in (("pd", "pdT"), ("ed", "encT"), ("pe", "peT")):
                gfix[(l, m)] = kv_pass(srcT_sb[src], l, m)

        resid = h_x
        for l in range(L):
            if l > 0:
                gath_sa = kv_pass(hT, l, "sa")
            q_sa = proj_pairs(hT, T[f"wq_{l}sa"], T[f"bq_{l}sa"], qTp, "qT")
            q_pd = proj_pairs(hT, T[f"wq_{l}pd"], T[f"bq_{l}pd"], qTp, "qT")
            z1 = zxp.tile([128, 2, 1024], F32, tag="zx")
            attn_stage(
                l, ("sa", "pd"), (q_sa, q_pd),
                (gath_sa, gfix[(l, "pd")]), True, resid, z1,
            )
            r1, r1T = ln_stage(z1, T[f"g1_{l}"], T[f"bb1_{l}"], (rTp, "rT"))

            q_ed = proj_pairs(r1T, T[f"wq_{l}ed"], T[f"bq_{l}ed"], qTp, "qT")
            q_pe = proj_pairs(r1T, T[f"wq_{l}pe"], T[f"bq_{l}pe"], qTp, "qT")
            z2 = zxp.tile([128, 2, 1024], F32, tag="zx")
            attn_stage(
                l, ("ed", "pe"), (q_ed, q_pe),
                (gfix[(l, "ed")], gfix[(l, "pe")]), False, r1, z2,
            )
            r2, r2T = ln_stage(z2, T[f"g2_{l}"], T[f"bb2_{l}"], (rTp, "rT"))

            z3 = zxp.tile([128, 2, 1024], F32, tag="zx")
            ffn(r2T, l, r2, z3)
            last = l == L - 1
            h_next, hT_next = ln_stage(
                z3, T[f"g3_{l}"], T[f"bb3_{l}"],
                None if last else (hTp, "hT"),
            )
            if last:
                nc.sync.dma_start(
                    out_d.ap().rearrange("(c p) d -> p c d", p=128), h_next[:]
                )
            else:
                hT = hT_next
                resid = h_next


_NC_CACHE = {}


def _get_nc():
    if "nc" not in _NC_CACHE:
        _NC_CACHE["nc"] = _build()
    return _NC_CACHE["nc"]


def _host_prep(x, encoder_out, pe_out, pd_out, params):
    """Build the 8 per-core input maps."""
    p = params

    def bf(a):
        return np.ascontiguousarray(np.asarray(a, np.float32)).astype(BF16)

    def f32(a):
        return np.ascontiguousarray(np.asarray(a, np.float32))

    shared = {}
    for l in range(L):
        for m in MHAS:
            pm = p[f"{m.replace('sa', 'self')}_attn" if m == "sa" else f"{m}_attn"]
            wq, wk, wv = pm["Wq"][l], pm["Wk"][l], pm["Wv"][l]  # (H, D, DK)
            shared[f"wq_{l}{m}"] = bf(np.transpose(wq, (1, 0, 2)).reshape(D, D))
            shared[f"wk_{l}{m}"] = bf(np.transpose(wk, (1, 0, 2)).reshape(D, D))
            shared[f"wv_{l}{m}"] = bf(np.transpose(wv, (1, 0, 2)).reshape(D, D))
            wh = np.asarray(pm["Wh"][l], np.float32)  # (H, DK, DK)
            shared[f"wh_{l}{m}"] = bf(
                np.transpose(wh.reshape(8, 2, DK, DK), (1, 2, 0, 3)).reshape(128, 512)
            )
            shared[f"wo_{l}{m}"] = bf(0.5 * np.asarray(pm["Wo"][l], np.float32))
            bq, bk, bh = pm["bq"][l], pm["bk"][l], pm["bh"][l]  # (H, DK)
            for nm, bv_ in (("bq", bq), ("bk", bk), ("bh", bh)):
                shared[f"{nm}_{l}{m}"] = f32(
                    np.transpose(np.asarray(bv_, np.float32).reshape(8, 2, DK), (1, 2, 0))
                    .reshape(128, 8)
                )
            shared[f"bv_{l}{m}"] = f32(pm["bv"][l]).reshape(1, D)
            shared[f"bo_{l}{m}"] = f32(0.5 * np.asarray(pm["bo"][l], np.float32)).reshape(1, D)
        shared[f"w1_{l}"] = bf(p["W1"][l])
        shared[f"b1c_{l}"] = f32(p["b1"][l]).reshape(32, 128).T.copy()
        shared[f"w2_{l}"] = bf(p["W2"][l])
        shared[f"b2r_{l}"] = f32(p["b2"][l]).reshape(1, D)
        for j, (g, b) in (
            (1, ("ln1_g", "ln1_b")),
            (2, ("ln2_g", "ln2_b")),
            (3, ("ln3_g", "ln3_b")),
        ):
            shared[f"g{j}_{l}"] = bf(np.asarray(p[g][l]).reshape(1, D))
            shared[f"bb{j}_{l}"] = bf(np.asarray(p[b][l]).reshape(1, D))

    x = np.asarray(x, np.float32)
    enc = np.asarray(encoder_out, np.float32)
    pe = np.asarray(pe_out, np.float32)
    pd = np.asarray(pd_out, np.float32)

    # AG key slot j -> absolute position
    agpos = np.empty(S, np.int64)
    for r in range(4):
        for u in range(SO):
            agpos[r * SO + u] = 128 * r + u if u < 128 else 128 * (7 - r) + (u - 128)

    in_maps = []
    for g in range(B):
        for c in range(4):
            lo = slice(128 * c, 128 * c + 128)
            hi = slice(128 * (7 - c), 128 * (8 - c))
            rows = np.r_[np.arange(128 * c, 128 * c + 128),
                         np.arange(128 * (7 - c), 128 * (8 - c))]
            m = dict(shared)
            xo = x[g][rows]
            m["x_own"] = f32(xo)
            m["xT"] = bf(xo.T)
            m["encT"] = bf(enc[g][rows].T)
            m["peT"] = bf(pe[g][rows].T)
            m["pdT"] = bf(pd[g][rows].T)
            qpos = np.concatenate(
                [np.arange(128 * c, 128 * c + 128),
                 np.arange(128 * (7 - c), 128 * (8 - c))]
            )  # (256,)
            mask = np.zeros((128, 8, SO), np.float32)
            for kb in range(8):
                kpos = agpos[kb * 128 : kb * 128 + 128]  # (128,)
                blocked = kpos[:, None] > qpos[None, :]  # (128, 256)
                mk = np.where(blocked, NEG, 0.0).astype(np.float32)
                if kb >= 4:
                    mk[:, 0:128] = 0.0  # low chunk never sees these blocks
                mask[:, kb, :] = mk
            m["maskT"] = mask.astype(BF16)
            in_maps.append(m)
    return in_maps


def kernel(x, encoder_out, pe_out, pd_out, params, _want_trace=False):
    nc = _get_nc()
    in_maps = _host_prep(x, encoder_out, pe_out, pd_out, params)
    res = bass_utils.run_bass_kernel_spmd(
        nc, in_maps, core_ids=list(range(NCORE)), trace=_want_trace
    )
    out = np.empty((B, S, D), np.float32)
    for g in range(B):
        for c in range(4):
            co = res.results[4 * g + c]["out"]
            out[g, 128 * c : 128 * c + 128] = co[0:128]
            out[g, 128 * (7 - c) : 128 * (8 - c)] = co[128:256]
    if _want_trace:
        kernel._last_results = res
    return out


# revision 8
# speedup vs baseline: 1.2757x; 1.2757x over previous
"""Trainium2 Bass kernel for a 2-layer, 4-attention-block transformer decoder.

Contract: kernel(**inputs) takes FULL inputs (x, encoder_out, pe_out, pd_out,
params) and returns the FULL (2, 1024, 1024) float32 output.

Distribution: 8 NeuronCores = 2 batch groups x 4-way sequence parallel.
Core (4g + c) owns batch g, query chunks {c, 7-c} (128 rows each).
Self/cross-attention K/V are computed position-sharded and AllGather'd
within each 4-core group; everything else is local. Causal attention is
block-causal (low chunk vs 512 keys, high chunk vs 1024 keys) with
per-core additive masks supplied as data, so one SPMD program serves all
cores. The per-head output projection Wh is folded into Wv host-side
(attn @ (V @ Wh) == (attn @ V) @ Wh); an extra all-ones column appended
to V yields the softmax denominator for free from the same matmuls.
"""

import sys

sys.path.insert(0, "/opt/trn_rl_repo")

import numpy as np
import ml_dtypes

import concourse.bass as bass
import concourse.mybir as mybir
import concourse.tile as tile
from concourse import bacc
from concourse import bass_utils

BF16 = ml_dtypes.bfloat16
F32 = mybir.dt.float32
B16 = mybir.dt.bfloat16

B, S, D, H, L, DF, DK = 2, 1024, 1024, 16, 2, 4096, 64
NCORE = 8
GROUPS = [[0, 1, 2, 3], [4, 5, 6, 7]]
SO = 256          # owned rows per core
VW = 66           # per-head v width in the gathered layout (64 + ones + pad)
SH_ROWS = 1024 + 4 * VW * 4  # 1024 k rows + 256*1056/256 v rows = 2080
MHAS = ("sa", "pd", "ed", "pe")
NEG = -30000.0

AF = mybir.ActivationFunctionType
ALU = mybir.AluOpType


def _fap(t_ap, extra_offset, dims):
    """AP on the same tensor with an explicit element offset + [step,count] dims."""
    return bass.AP(
        tensor=t_ap.tensor, offset=t_ap.offset + extra_offset,
        ap=[list(d) for d in dims],
    )


def _bcast_rows(ap, p):
    """View a (1, N) DRAM AP as (p, N) with partition stride 0 (DMA broadcast)."""
    return bass.AP(tensor=ap.tensor, offset=ap.offset, ap=[[0, p]] + list(ap.ap[1:]))


def _build(zero_bias, ln_id):
    nc = bacc.Bacc("TRN2", target_bir_lowering=False, debug=False, num_devices=NCORE)

    T = {}

    def din(name, shape, dt):
        T[name] = nc.dram_tensor(name, list(shape), dt, kind="ExternalInput")
        return T[name]

    din("xT", (D, SO), B16)
    din("x_own", (SO, D), F32)
    din("encT", (D, SO), B16)
    din("peT", (D, SO), B16)
    din("pdT", (D, SO), B16)
    din("maskP", (128, 4, 512), B16)
    for l in range(L):
        for m in MHAS:
            din(f"wq_{l}{m}", (D, D), B16)
            din(f"wk_{l}{m}", (D, D), B16)
            din(f"wv_{l}{m}", (D, D), B16)   # Wv @ Wh folded, head-major cols
            din(f"wo_{l}{m}", (D, D), B16)   # pre-scaled by 0.5
            if not zero_bias:
                din(f"bq_{l}{m}", (128, 8), F32)
                din(f"bk_{l}{m}", (128, 8), F32)
                din(f"bh_{l}{m}", (128, 8), F32)
                din(f"bv_{l}{m}", (1, D), F32)   # bv @ Wh folded
                din(f"bo_{l}{m}", (1, D), F32)   # pre-scaled by 0.5
        din(f"w1_{l}", (D, DF), B16)
        din(f"w2_{l}", (DF, D), B16)
        if not zero_bias:
            din(f"b1c_{l}", (128, 32), F32)
            din(f"b2r_{l}", (1, D), F32)
        if not ln_id:
            for j in (1, 2, 3):
                din(f"g{j}_{l}", (1, D), B16)
                din(f"bb{j}_{l}", (1, D), B16)
    out_d = nc.dram_tensor("out", [SO, D], F32, kind="ExternalOutput")

    with tile.TileContext(nc) as tc:
        _program(nc, tc, T, out_d, zero_bias, ln_id)

    nc.compile()
    return nc


def _program(nc, tc, T, out_d, zero_bias, ln_id):
    import contextlib

    ctx = contextlib.ExitStack()
    with ctx:
        ps = ctx.enter_context(tc.tile_pool(name="ps", bufs=8, space="PSUM"))
        wp = ctx.enter_context(tc.tile_pool(name="wp", bufs=10))
        wsm = ctx.enter_context(tc.tile_pool(name="wsm", bufs=3))
        cst = ctx.enter_context(tc.tile_pool(name="cst", bufs=1))
        hTp = ctx.enter_context(tc.tile_pool(name="hTp", bufs=2))
        hxp = ctx.enter_context(tc.tile_pool(name="hxp", bufs=1))
        rTp = ctx.enter_context(tc.tile_pool(name="rTp", bufs=2))
        zxp = ctx.enter_context(tc.tile_pool(name="zxp", bufs=3))
        qTp = ctx.enter_context(tc.tile_pool(name="qTp", bufs=2))
        kTp = ctx.enter_context(tc.tile_pool(name="kTp", bufs=1))
        vtp = ctx.enter_context(tc.tile_pool(name="vtp", bufs=1))
        pTp = ctx.enter_context(tc.tile_pool(name="pTp", bufs=4))
        c2p = ctx.enter_context(tc.tile_pool(name="c2p", bufs=2))
        a1p = ctx.enter_context(tc.tile_pool(name="a1p", bufs=1))
        sev = ctx.enter_context(tc.tile_pool(name="sev", bufs=4))
        seb = ctx.enter_context(tc.tile_pool(name="seb", bufs=4))
        kev = ctx.enter_context(tc.tile_pool(name="kev", bufs=2))
        vev = ctx.enter_context(tc.tile_pool(name="vev", bufs=2))
        bnp = ctx.enter_context(tc.tile_pool(name="bnp", bufs=4))
        shp = ctx.enter_context(tc.tile_pool(name="shp", bufs=2, space="DRAM"))
        gap = ctx.enter_context(tc.tile_pool(name="gap", bufs=8, space="DRAM"))
        if not zero_bias:
            b8p = ctx.enter_context(tc.tile_pool(name="b8p", bufs=6))
            brp = ctx.enter_context(tc.tile_pool(name="brp", bufs=2))
            b1p = ctx.enter_context(tc.tile_pool(name="b1p", bufs=2))
        if not ln_id:
            gbp = ctx.enter_context(tc.tile_pool(name="gbp", bufs=2))

        # ---- constants ----
        ident = cst.tile([128, 128], F32, tag="ident")
        from concourse.masks import make_identity

        make_identity(nc, ident[:])
        eps_t = cst.tile([128, 1], F32, tag="eps")
        nc.vector.memset(eps_t[:], 1e-5)
        ones1 = cst.tile([1, 128], F32, tag="ones1")
        nc.vector.memset(ones1[:], 1.0)
        masks = cst.tile([128, 4, 512], B16, tag="masks")
        nc.sync.dma_start(masks[:], T["maskP"].ap())
        srcT_sb = {}
        for nm in ("encT", "peT", "pdT"):
            t = cst.tile([128, 8, SO], B16, tag=nm)
            nc.sync.dma_start(t[:], T[nm].ap().rearrange("(ks p) c -> p ks c", p=128))
            srcT_sb[nm] = t

        def load_w_slices(w_d, n=8):
            ts_ = []
            wv = w_d.ap().rearrange("(ks p) n -> ks p n", p=128)
            for k in range(n):
                wt = wp.tile([128, 1024], B16, tag="w", name=f"w_{k}")
                nc.sync.dma_start(wt[:], wv[k])
                ts_.append(wt)
            return ts_

        def proj_pairs(srcT, w_d, b8_d, outpool, tag):
            """pairs(hp) = (W[:, hp*128:+128]).T @ src  (+bias); -> (128, 8, 256) bf16."""
            out_sb = outpool.tile([128, 8, SO], B16, tag=tag)
            b_sb = None
            if b8_d is not None:
                b_sb = b8p.tile([128, 8], F32, tag="b8")
                nc.sync.dma_start(b_sb[:], b8_d.ap())
            wt = load_w_slices(w_d)
            for half in range(2):
                pss = [
                    ps.tile([128, 512], F32, tag="ps", name=f"ps_pp{i}")
                    for i in range(4)
                ]
                for ks in range(8):
                    for i in range(4):
                        hp = 4 * half + i
                        nc.tensor.matmul(
                            pss[i][:, :SO],
                            wt[ks][:, hp * 128 : hp * 128 + 128],
                            srcT[:, ks, :],
                            start=(ks == 0),
                            stop=(ks == 7),
                        )
                for i in range(4):
                    hp = 4 * half + i
                    if b_sb is not None:
                        nc.scalar.activation(
                            out_sb[:, hp, :], pss[i][:, :SO], AF.Identity,
                            bias=b_sb[:, hp : hp + 1],
                        )
                    else:
                        nc.scalar.activation(
                            out_sb[:, hp, :], pss[i][:, :SO], AF.Identity
                        )
            return out_sb

        def proj_v(srcT, w_d, brow_d):
            """v' = src @ W (+brow), written in the interleaved 66-wide layout
            with the ones column baked in; -> (128, 2, 1056) bf16."""
            out_sb = vev.tile([128, 2, 16 * VW], B16, tag="vev")
            o4 = out_sb[:].rearrange("p c (hh w) -> p c hh w", w=VW)
            nc.vector.memset(o4[:, :, :, 64:VW], 1.0)
            br = None
            if brow_d is not None:
                br = brp.tile([1, 1024], F32, tag="brow")
                nc.sync.dma_start(br[:], brow_d.ap())
            wt = load_w_slices(w_d)
            pss = [
                ps.tile([128, 512], F32, tag="ps", name=f"ps_pv{i}") for i in range(4)
            ]
            for ks in range(8):
                for c in range(2):
                    for ns in range(2):
                        nc.tensor.matmul(
                            pss[c * 2 + ns],
                            srcT[:, ks, c * 128 : c * 128 + 128],
                            wt[ks][:, ns * 512 : ns * 512 + 512],
                            start=(ks == 0),
                            stop=(ks == 7) if br is None else False,
                        )
            for c in range(2):
                for ns in range(2):
                    if br is not None:
                        nc.tensor.matmul(
                            pss[c * 2 + ns], ones1[:, :],
                            br[0:1, ns * 512 : ns * 512 + 512],
                            start=False, stop=True, skip_group_check=True,
                        )
                    nc.scalar.activation(
                        o4[:, c, 8 * ns : 8 * ns + 8, 0:64],
                        pss[c * 2 + ns][:].rearrange("p (hh w) -> p hh w", w=64),
                        AF.Identity,
                    )
            return out_sb

        def kv_pass(srcT, l, m):
            shard = shp.tile([SH_ROWS, SO], B16, tag="shard")
            gath = gap.tile([4 * SH_ROWS, SO], B16, tag="gath")
            k_sb = proj_pairs(
                srcT, T[f"wk_{l}{m}"],
                None if zero_bias else T[f"bk_{l}{m}"], kev, "kev",
            )
            nc.sync.dma_start(
                _fap(shard[:], 0, [[256, 128], [128 * 256, 8], [1, 256]]), k_sb[:]
            )
            v_sb = proj_v(
                srcT, T[f"wv_{l}{m}"], None if zero_bias else T[f"bv_{l}{m}"]
            )
            for c in range(2):
                nc.sync.dma_start(
                    _fap(
                        shard[:], 1024 * 256 + c * 128 * (16 * VW),
                        [[16 * VW, 128], [1, 16 * VW]],
                    ),
                    v_sb[:, c, :],
                )
            nc.gpsimd.collective_compute(
                "AllGather", ALU.bypass, replica_groups=GROUPS,
                ins=[shard[:].opt()], outs=[gath[:].opt()],
            )
            return gath

        def attn_stage(l, mhas, qts, gaths, masked, resid_x, zx_out):
            for mi, m in enumerate(mhas):
                gath = gaths[mi]
                qT = qts[mi]
                kT = kTp.tile([128, 8, 1024], B16, tag="kT")
                kT4 = kT[:].rearrange("p hp (r u) -> p hp r u", r=4)
                for r in range(4):
                    nc.sync.dma_start(
                        kT4[:, :, r],
                        _fap(
                            gath[:], r * SH_ROWS * 256,
                            [[256, 128], [128 * 256, 8], [1, 256]],
                        ),
                    )
                vt = vtp.tile([128, 8, 16 * VW], B16, tag="vt")
                for kb in range(8):
                    r, pos0 = kb // 2, (kb % 2) * 128
                    nc.sync.dma_start(
                        vt[:, kb, :],
                        _fap(
                            gath[:],
                            r * SH_ROWS * 256 + 1024 * 256 + pos0 * (16 * VW),
                            [[16 * VW, 128], [1, 16 * VW]],
                        ),
                    )
                bh_sb = None
                if not zero_bias:
                    bh_sb = b8p.tile([128, 8], F32, tag="b8")
                    nc.sync.dma_start(bh_sb[:], T[f"bh_{l}{m}"].ap())
                cat2 = c2p.tile([128, 8, SO], B16, tag="cat2")

                for hp in range(8):
                    for hi in range(2):
                        h = 2 * hp + hi
                        r0 = 64 * hi
                        ctx_ps = ps.tile([128, 512], F32, tag="ps", name="ps_ctx")
                        for j in range(4):
                            s_ps = ps.tile([128, 512], F32, tag="ps", name="ps_s")
                            if masked:
                                wdt = 512 if j < 2 else 256
                                for t in range(2):
                                    kb = 2 * j + t
                                    if j < 2:
                                        cb = 256 * t
                                        nc.tensor.matmul(
                                            s_ps[:, cb + 128 : cb + 256],
                                            kT[r0 : r0 + 64, hp, kb * 128 : kb * 128 + 128],
                                            qT[r0 : r0 + 64, hp, 128:256],
                                            start=True, stop=True,
                                            skip_group_check=True,
                                        )
                                        nc.tensor.matmul(
                                            s_ps[:, cb : cb + 128],
                                            kT[r0 : r0 + 64, hp, kb * 128 : kb * 128 + 128],
                                            qT[r0 : r0 + 64, hp, 0:128],
                                            start=True, stop=True,
                                            skip_group_check=True,
                                        )
                                    else:
                                        nc.tensor.matmul(
                                            s_ps[:, 128 * t : 128 * t + 128],
                                            kT[r0 : r0 + 64, hp, kb * 128 : kb * 128 + 128],
                                            qT[r0 : r0 + 64, hp, 128:256],
                                            start=True, stop=True,
                                            skip_group_check=True,
                                        )
                                nc.vector.tensor_tensor(
                                    s_ps[:, 0:wdt], s_ps[:, 0:wdt],
                                    masks[:, j, 0:wdt], ALU.add,
                                )
                                pT = pTp.tile([128, 512], B16, tag="pT")
                                nc.scalar.activation(
                                    pT[:, 0:wdt], s_ps[:, 0:wdt], AF.Exp, scale=0.125
                                )
                                for t in range(2):
                                    kb = 2 * j + t
                                    if j < 2:
                                        nc.tensor.matmul(
                                            ctx_ps[0:65, 0:256],
                                            vt[:, kb, h * VW : h * VW + 65],
                                            pT[:, 256 * t : 256 * t + 256],
                                            start=(j == 0 and t == 0),
                                            stop=(j == 3 and t == 1),
                                            skip_group_check=True,
                                        )
                                    else:
                                        nc.tensor.matmul(
                                            ctx_ps[0:65, 128:256],
                                            vt[:, kb, h * VW : h * VW + 65],
                                            pT[:, 128 * t : 128 * t + 128],
                                            start=False,
                                            stop=(j == 3 and t == 1),
                                            skip_group_check=True,
                                        )
                            else:
                                for t in range(2):
                                    kb = 2 * j + t
                                    nc.tensor.matmul(
                                        s_ps[:, 256 * t : 256 * t + 256],
                                        kT[r0 : r0 + 64, hp, kb * 128 : kb * 128 + 128],
                                        qT[r0 : r0 + 64, hp, :],
                                        start=True, stop=True, skip_group_check=True,
                                    )
                                pT = pTp.tile([128, 512], B16, tag="pT")
                                nc.scalar.activation(
                                    pT[:], s_ps[:], AF.Exp, scale=0.125
                                )
                                for t in range(2):
                                    kb = 2 * j + t
                                    nc.tensor.matmul(
                                        ctx_ps[0:65, 0:256],
                                        vt[:, kb, h * VW : h * VW + 65],
                                        pT[:, 256 * t : 256 * t + 256],
                                        start=(j == 0 and t == 0),
                                        stop=(j == 3 and t == 1),
                                        skip_group_check=True,
                                    )
                        se_sb = sev.tile([1, SO], F32, tag="sev")
                        nc.scalar.activation(
                            se_sb[:], ctx_ps[64:65, 0:256], AF.Identity
                        )
                        se_iv = sev.tile([1, SO], F32, tag="sev")
                        nc.vector.reciprocal_approx_fast(out=se_iv[:], in_=se_sb[:])
                        seb_t = seb.tile([64, SO], F32, tag="seb")
                        nc.gpsimd.partition_broadcast(seb_t[:], se_iv[:])
                        out_sl = cat2[r0 : r0 + 64, hp, :]
                        nc.vector.tensor_tensor(
                            out_sl, ctx_ps[0:64, 0:256], seb_t[:], ALU.mult
                        )
                        if bh_sb is not None:
                            nc.vector.tensor_tensor(
                                out_sl, out_sl,
                                bh_sb[r0 : r0 + 64, hp : hp + 1].to_broadcast((64, SO)),
                                ALU.add,
                            )

                # Wo (pre-scaled by 0.5 host-side) + accumulate into zx_out
                wo_t = load_w_slices(T[f"wo_{l}{m}"])
                bo_r = None
                if not zero_bias:
                    bo_r = brp.tile([1, 1024], F32, tag="brow")
                    nc.sync.dma_start(bo_r[:], T[f"bo_{l}{m}"].ap())
                for c in range(2):
                    for ns in range(2):
                        o_ps = ps.tile([128, 512], F32, tag="ps", name="ps_o")
                        for kp in range(8):
                            nc.tensor.matmul(
                                o_ps,
                                cat2[:, kp, c * 128 : c * 128 + 128],
                                wo_t[kp][:, ns * 512 : ns * 512 + 512],
                                start=(kp == 0),
                                stop=(kp == 7) if bo_r is None else False,
                            )
                        if bo_r is not None:
                            nc.tensor.matmul(
                                o_ps, ones1[:, :], bo_r[0:1, ns * 512 : ns * 512 + 512],
                                start=False, stop=True, skip_group_check=True,
                            )
                        sl = slice(ns * 512, ns * 512 + 512)
                        if mi == 0:
                            nc.vector.tensor_tensor(
                                zx_out[:, c, sl], o_ps, resid_x[:, c, sl], ALU.add
                            )
                        else:
                            nc.vector.tensor_tensor(
                                zx_out[:, c, sl], o_ps, zx_out[:, c, sl], ALU.add
                            )

        def ln_stage(z_x, lnj, l, transpose_to=None):
            if not ln_id:
                g_b = gbp.tile([128, 1024], B16, tag="gb")
                nc.sync.dma_start(g_b[:], _bcast_rows(T[f"g{lnj}_{l}"].ap(), 128))
                b_b = gbp.tile([128, 1024], B16, tag="gb")
                nc.sync.dma_start(b_b[:], _bcast_rows(T[f"bb{lnj}_{l}"].ap(), 128))
            for c in range(2):
                zc = z_x[:, c, :]
                st = bnp.tile([128, 2, 6], F32, tag="bnst")
                for sg in range(2):
                    nc.vector.bn_stats(st[:, sg, :], zc[:, sg * 512 : sg * 512 + 512])
                mv = bnp.tile([128, 2], F32, tag="bnmv")
                nc.vector.bn_aggr(mv[:], st[:])
                rstd = bnp.tile([128, 1], F32, tag="rstd")
                nc.scalar.activation(rstd[:], mv[:, 1:2], AF.Sqrt, bias=eps_t[:, :])
                nc.vector.reciprocal(rstd[:], rstd[:])
                nc.vector.tensor_scalar(
                    zc, zc, mv[:, 0:1], rstd[:, 0:1], ALU.subtract, ALU.mult
                )
                if not ln_id:
                    nc.vector.tensor_tensor(zc, zc, g_b[:], ALU.mult)
                    nc.vector.tensor_tensor(zc, zc, b_b[:], ALU.add)
            rT = None
            if transpose_to is not None:
                pool, tag = transpose_to
                rT = pool.tile([128, 8, SO], B16, tag=tag)
                for c in range(2):
                    for ks in range(8):
                        tp = ps.tile([128, 512], F32, tag="ps", name="ps_t")
                        nc.tensor.transpose(
                            tp[:, 0:128], z_x[:, c, ks * 128 : ks * 128 + 128], ident[:]
                        )
                        nc.scalar.activation(
                            rT[:, ks, c * 128 : c * 128 + 128], tp[:, 0:128],
                            AF.Identity,
                        )
            return z_x, rT

        def ffn(r2T, l, resid_x, zx_out):
            b1c = None
            if not zero_bias:
                b1c = b1p.tile([128, 32], F32, tag="b1c")
                nc.sync.dma_start(b1c[:], T[f"b1c_{l}"].ap())
            a1 = a1p.tile([128, 32, SO], B16, tag="a1")
            w1v = T[f"w1_{l}"].ap().rearrange("(ks p) n -> ks p n", p=128)
            for dtg in range(8):
                pss = [
                    ps.tile([128, 512], F32, tag="ps", name=f"ps_f{i}")
                    for i in range(4)
                ]
                for ks in range(8):
                    wt = wsm.tile([128, 512], B16, tag="wsm")
                    nc.sync.dma_start(wt[:], w1v[ks, :, dtg * 512 : dtg * 512 + 512])
                    for d4 in range(4):
                        nc.tensor.matmul(
                            pss[d4][:, :SO],
                            wt[:, d4 * 128 : d4 * 128 + 128],
                            r2T[:, ks, :],
                            start=(ks == 0), stop=(ks == 7),
                        )
                for d4 in range(4):
                    dt = dtg * 4 + d4
                    if b1c is not None:
                        nc.scalar.activation(
                            a1[:, dt, :], pss[d4][:, :SO], AF.Relu,
                            bias=b1c[:, dt : dt + 1],
                        )
                    else:
                        nc.scalar.activation(a1[:, dt, :], pss[d4][:, :SO], AF.Relu)
            w2v = T[f"w2_{l}"].ap().rearrange("(ks p) n -> ks p n", p=128)
            fps = [
                ps.tile([128, 512], F32, tag="ps", name=f"ps_ff{i}") for i in range(4)
            ]
            for ks in range(32):
                w2t = wp.tile([128, 1024], B16, tag="w", name="w2t")
                nc.sync.dma_start(w2t[:], w2v[ks])
                for c in range(2):
                    for ns in range(2):
                        nc.tensor.matmul(
                            fps[c * 2 + ns],
                            a1[:, ks, c * 128 : c * 128 + 128],
                            w2t[:, ns * 512 : ns * 512 + 512],
                            start=(ks == 0),
                            stop=(ks == 31) if zero_bias else False,
                        )
            b2r = None
            if not zero_bias:
                b2r = brp.tile([1, 1024], F32, tag="brow")
                nc.sync.dma_start(b2r[:], T[f"b2r_{l}"].ap())
            for c in range(2):
                for ns in range(2):
                    if b2r is not None:
                        nc.tensor.matmul(
                            fps[c * 2 + ns], ones1[:, :],
                            b2r[0:1, ns * 512 : ns * 512 + 512],
                            start=False, stop=True, skip_group_check=True,
                        )
                    sl = slice(ns * 512, ns * 512 + 512)
                    nc.vector.tensor_tensor(
                        zx_out[:, c, sl], fps[c * 2 + ns], resid_x[:, c, sl], ALU.add
                    )

        # ---- program ----
        hT = hTp.tile([128, 8, SO], B16, tag="hT")
        nc.sync.dma_start(hT[:], T["xT"].ap().rearrange("(ks p) c -> p ks c", p=128))
        h_x = hxp.tile([128, 2, 1024], F32, tag="hx")
        nc.sync.dma_start(h_x[:], T["x_own"].ap().rearrange("(c p) d -> p c d", p=128))

        gath_sa = kv_pass(hT, 0, "sa")
        gfix = {}
        for l in range(L):
            for m, src in (("pd", "pdT"), ("ed", "encT"), ("pe", "peT")):
                gfix[(l, m)] = kv_pass(srcT_sb[src], l, m)

        def bias_or_none(name):
            return None if zero_bias else T[name]

        resid = h_x
        for l in range(L):
            if l > 0:
                gath_sa = kv_pass(hT, l, "sa")
            q_sa = proj_pairs(hT, T[f"wq_{l}sa"], bias_or_none(f"bq_{l}sa"), qTp, "qT")
            q_pd = proj_pairs(hT, T[f"wq_{l}pd"], bias_or_none(f"bq_{l}pd"), qTp, "qT")
            z1 = zxp.tile([128, 2, 1024], F32, tag="zx")
            attn_stage(
                l, ("sa", "pd"), (q_sa, q_pd),
                (gath_sa, gfix[(l, "pd")]), True, resid, z1,
            )
            r1, r1T = ln_stage(z1, 1, l, (rTp, "rT"))

            q_ed = proj_pairs(r1T, T[f"wq_{l}ed"], bias_or_none(f"bq_{l}ed"), qTp, "qT")
            q_pe = proj_pairs(r1T, T[f"wq_{l}pe"], bias_or_none(f"bq_{l}pe"), qTp, "qT")
            z2 = zxp.tile([128, 2, 1024], F32, tag="zx")
            attn_stage(
                l, ("ed", "pe"), (q_ed, q_pe),
                (gfix[(l, "ed")], gfix[(l, "pe")]), False, r1, z2,
            )
            r2, r2T = ln_stage(z2, 2, l, (rTp, "rT"))

            z3 = zxp.tile([128, 2, 1024], F32, tag="zx")
            ffn(r2T, l, r2, z3)
            last = l == L - 1
            h_next, hT_next = ln_stage(z3, 3, l, None if last else (hTp, "hT"))
            if last:
                nc.sync.dma_start(
                    out_d.ap().rearrange("(c p) d -> p c d", p=128), h_next[:]
                )
            else:
                hT = hT_next
                resid = h_next


_NC_CACHE = {}


def _get_nc(zero_bias, ln_id):
    key = (zero_bias, ln_id)
    if key not in _NC_CACHE:
        _NC_CACHE[key] = _build(zero_bias, ln_id)
    return _NC_CACHE[key]


def _host_prep(x, encoder_out, pe_out, pd_out, params, zero_bias, ln_id):
    p = params

    def bf(a):
        return np.ascontiguousarray(np.asarray(a, np.float32)).astype(BF16)

    def f32(a):
        return np.ascontiguousarray(np.asarray(a, np.float32))

    shared = {}
    for l in range(L):
        for m in MHAS:
            pm = p["self_attn" if m == "sa" else f"{m}_attn"]
            wq = np.asarray(pm["Wq"][l], np.float32)
            wk = np.asarray(pm["Wk"][l], np.float32)
            wv = np.asarray(pm["Wv"][l], np.float32)
            wh = np.asarray(pm["Wh"][l], np.float32)
            shared[f"wq_{l}{m}"] = bf(np.transpose(wq, (1, 0, 2)).reshape(D, D))
            shared[f"wk_{l}{m}"] = bf(np.transpose(wk, (1, 0, 2)).reshape(D, D))
            wvh = np.einsum("hde,hef->dhf", wv, wh)  # fold Wh into Wv
            shared[f"wv_{l}{m}"] = bf(wvh.reshape(D, D))
            shared[f"wo_{l}{m}"] = bf(0.5 * np.asarray(pm["Wo"][l], np.float32))
            if not zero_bias:
                bq, bk, bh = pm["bq"][l], pm["bk"][l], pm["bh"][l]
                for nm, bv_ in (("bq", bq), ("bk", bk), ("bh", bh)):
                    shared[f"{nm}_{l}{m}"] = f32(
                        np.transpose(
                            np.asarray(bv_, np.float32).reshape(8, 2, DK), (1, 2, 0)
                        ).reshape(128, 8)
                    )
                bvh = np.einsum("he,hef->hf", np.asarray(pm["bv"][l], np.float32), wh)
                shared[f"bv_{l}{m}"] = f32(bvh).reshape(1, D)
                shared[f"bo_{l}{m}"] = f32(
                    0.5 * np.asarray(pm["bo"][l], np.float32)
                ).reshape(1, D)
        shared[f"w1_{l}"] = bf(p["W1"][l])
        shared[f"w2_{l}"] = bf(p["W2"][l])
        if not zero_bias:
            shared[f"b1c_{l}"] = f32(p["b1"][l]).reshape(32, 128).T.copy()
            shared[f"b2r_{l}"] = f32(p["b2"][l]).reshape(1, D)
        if not ln_id:
            for j, (g, b) in (
                (1, ("ln1_g", "ln1_b")),
                (2, ("ln2_g", "ln2_b")),
                (3, ("ln3_g", "ln3_b")),
            ):
                shared[f"g{j}_{l}"] = bf(np.asarray(p[g][l]).reshape(1, D))
                shared[f"bb{j}_{l}"] = bf(np.asarray(p[b][l]).reshape(1, D))

    x = np.asarray(x, np.float32)
    enc = np.asarray(encoder_out, np.float32)
    pe = np.asarray(pe_out, np.float32)
    pd = np.asarray(pd_out, np.float32)

    agpos = np.empty(S, np.int64)
    for r in range(4):
        for u in range(SO):
            agpos[r * SO + u] = 128 * r + u if u < 128 else 128 * (7 - r) + (u - 128)

    in_maps = []
    for g in range(B):
        for c in range(4):
            rows = np.r_[np.arange(128 * c, 128 * c + 128),
                         np.arange(128 * (7 - c), 128 * (8 - c))]
            m = dict(shared)
            xo = x[g][rows]
            m["x_own"] = f32(xo)
            m["xT"] = bf(xo.T)
            m["encT"] = bf(enc[g][rows].T)
            m["peT"] = bf(pe[g][rows].T)
            m["pdT"] = bf(pd[g][rows].T)
            qpos = rows
            mask = np.zeros((128, 8, SO), np.float32)
            for kb in range(8):
                kpos = agpos[kb * 128 : kb * 128 + 128]
                mk = np.where(kpos[:, None] > qpos[None, :], NEG, 0.0)
                if kb >= 4:
                    mk[:, 0:128] = 0.0
                mask[:, kb, :] = mk
            # packed layout: j<2 -> [kb=2j (256) | kb=2j+1 (256)]
            #                j>=2 -> [kb=2j hi (128) | kb=2j+1 hi (128) | zeros]
            maskp = np.zeros((128, 4, 512), np.float32)
            for j in range(2):
                maskp[:, j, 0:256] = mask[:, 2 * j, :]
                maskp[:, j, 256:512] = mask[:, 2 * j + 1, :]
            for j in range(2, 4):
                maskp[:, j, 0:128] = mask[:, 2 * j, 128:256]
                maskp[:, j, 128:256] = mask[:, 2 * j + 1, 128:256]
            m["maskP"] = maskp.astype(BF16)
            in_maps.append(m)
    return in_maps


def kernel(x, encoder_out, pe_out, pd_out, params, _want_trace=False):
    p = params

    def _allz(*arrs):
        return all(not np.any(np.asarray(a)) for a in arrs)

    zero_bias = _allz(
        p["b1"], p["b2"],
        *[p[k + "_attn"]["b" + t] for k in ("self", "pd", "ed", "pe")
          for t in ("q", "k", "v", "h", "o")],
    )
    ln_id = _allz(
        p["ln1_b"], p["ln2_b"], p["ln3_b"],
        np.asarray(p["ln1_g"]) - 1, np.asarray(p["ln2_g"]) - 1,
        np.asarray(p["ln3_g"]) - 1,
    )
    nc = _get_nc(zero_bias, ln_id)
    in_maps = _host_prep(x, encoder_out, pe_out, pd_out, params, zero_bias, ln_id)
    res = bass_utils.run_bass_kernel_spmd(
        nc, in_maps, core_ids=list(range(NCORE)), trace=_want_trace
    )
    out = np.empty((B, S, D), np.float32)
    for g in range(B):
        for c in range(4):
            co = res.results[4 * g + c]["out"]
            out[g, 128 * c : 128 * c + 128] = co[0:128]
            out[g, 128 * (7 - c) : 128 * (8 - c)] = co[128:256]
    if _want_trace:
        kernel._last_results = res
    return out
